# revision 5
# baseline (speedup 1.0000x reference)
"""Trainium2 Bass kernel for the NeuralODE layer (dopri5 fixed-step, 8 steps).

Strategy: pure data parallel over 8 NeuronCores (batch 16384 -> 2048/core),
feature-on-partition layout ([512 feat -> 4 blocks of 128][batch cols]),
two half-batches of 1024 columns per core so state stays SBUF-resident.

Precision/engine plan (v2, fp8 DoubleRow):
- The 144 inner [*,512]x[512,512] matmuls run in fp8 e4m3 with
  MatmulPerfMode.DoubleRow (2 K-tiles of 128 per pass, 0.5 cyc/row -> ~4x the
  fp16 rate). Weights are host-quantized to fp8 at scale 16; activations are
  unscaled fp8 (tanh outputs in [-1,1], state ~N(0,1.4)).
- tanh drains on the Activation engine write fp8 directly (scale=1/16 folds
  the weight scale, bias folds b1/b2 plus the analytically-propagated b3
  correction per stage).
- b3 is never added on-chip: k-tiles hold m = h2@W3 only. Its contribution is
  folded on the host into per-stage layer-1 biases (beta_j * b3 @ W1) and into
  the once-per-step state update (+h*b3 column).
- k drains (layer 3) go to the Pool engine as scaled copies psum->fp8.
- RK linear combinations (P_j partials fp16, cf fp16, state fp32) are split
  across Pool and DVE; the stage-gating ops (x8_j, s8) are chunked so the next
  stage's matmuls start early.
"""

import numpy as np
import ml_dtypes

import concourse.bacc as bacc
import concourse.tile as tile
import concourse.mybir as mybir
from concourse.bass_utils import run_bass_kernel_spmd

F32 = mybir.dt.float32
F16 = mybir.dt.float16
F8 = mybir.dt.float8e4
AF = mybir.ActivationFunctionType
OP = mybir.AluOpType
DR = mybir.MatmulPerfMode.DoubleRow

N_CORES = 8
B, IN_DIM, HID = 16384, 256, 512
BSH = B // N_CORES          # 2048 batch rows per core
HALF = 1024                 # batch columns per half-integration
NSTEPS = 8
H = 0.1 * 1 / 8             # dt per solver step
WS = 16.0                   # fp8 weight scale

# Dormand-Prince tableau
_A = (
    (1 / 5,),
    (3 / 40, 9 / 40),
    (44 / 45, -56 / 15, 32 / 9),
    (19372 / 6561, -25360 / 2187, 64448 / 6561, -212 / 729),
    (9017 / 3168, -355 / 33, 46732 / 5247, 49 / 176, -5103 / 18656),
)
_B = (35 / 384, 0.0, 500 / 1113, 125 / 192, -2187 / 6784, 11 / 84)

KB = HID // 128             # 4 partition blocks of the feature dim
KBP = IN_DIM // 128         # 2 partition blocks for the input dim
NC_CHUNK = 512              # moving-operand columns per matmul (1 PSUM bank)
CPH = HALF // NC_CHUNK      # matmul chunks per half (2)

# engine split for the RK scatter/cf ops: (st, j) -> "pool"/"dve"
# (GPSIMD/Pool cannot read PSUM, so DVE owns the k drains + gating; Pool owns
# the SBUF-only scatter/cf/state traffic)
_SCATTER_ENG = {
    (0, 3): "dve", (0, 4): "dve", (0, 5): "dve", (0, 6): "dve",
    (1, 4): "dve", (1, 5): "dve", (1, 6): "dve",
    (2, 5): "dve", (2, 6): "dve",
    (3, 6): "dve",
}
_CF_ENG = {0: "pool", 2: "dve", 3: "dve", 4: "dve"}


def _dr_layer(nc, pp, w_t, x_t, drain):
    """acc[mb] = x @ W via fp8 DoubleRow matmuls; drain(mb, acc) consumes.

    x_t: [128, KB, HALF] fp8; w_t: [128, KB, 512] fp8 (block (kb, mb) at
    [:, kb, mb*128:(mb+1)*128]).
    """
    for mb in range(4):
        acc = pp.tile([128, HALF], F32, tag="psum", name="acc")
        for c in range(CPH):
            for p in range(2):
                nc.tensor.matmul(
                    acc[:, c * NC_CHUNK:(c + 1) * NC_CHUNK],
                    w_t[:, 2 * p:2 * p + 2, mb * 128:(mb + 1) * 128],
                    x_t[:, 2 * p:2 * p + 2, c * NC_CHUNK:(c + 1) * NC_CHUNK],
                    start=(p == 0), stop=(p == 1),
                    perf_mode=DR,
                )
        drain(mb, acc)


def build_nc(n_steps=NSTEPS):
    nc = bacc.Bacc("TRN2", target_bir_lowering=False, debug=False,
                   num_devices=N_CORES)

    yT = nc.declare_dram_parameter("yT", [HID, BSH], F32, isOutput=False)
    uT = nc.declare_dram_parameter("uT", [2 * IN_DIM, BSH], F16, isOutput=False)
    w1d = nc.declare_dram_parameter("w1", [HID, HID], F8, isOutput=False)
    w2d = nc.declare_dram_parameter("w2", [HID, HID], F8, isOutput=False)
    w3d = nc.declare_dram_parameter("w3", [HID, HID], F8, isOutput=False)
    wpd = nc.declare_dram_parameter("wp", [2 * IN_DIM, HID], F16, isOutput=False)
    bpd = nc.declare_dram_parameter("bp", [128, 4], F32, isOutput=False)
    b1ed = nc.declare_dram_parameter("b1e", [128, 24], F32, isOutput=False)
    b2d = nc.declare_dram_parameter("b2", [128, 4], F32, isOutput=False)
    hb3d = nc.declare_dram_parameter("hb3", [128, 4], F32, isOutput=False)
    outT = nc.declare_dram_parameter("outT", [HID, BSH], F32, isOutput=True)

    with tile.TileContext(nc) as tc:
        with (
            tc.tile_pool(name="wpool", bufs=1) as wp_,
            tc.tile_pool(name="spool", bufs=1) as sp,
            tc.tile_pool(name="pp", bufs=4, space="PSUM") as pp,
        ):
            # ---- resident weights/biases ----
            wpt = wp_.tile([128, 2 * KBP * 512], F16, tag="wp")
            for kb in range(2 * KBP):
                nc.gpsimd.dma_start(wpt[:, kb * 512:(kb + 1) * 512],
                                    wpd[kb * 128:(kb + 1) * 128, :])
            bpt = wp_.tile([128, 4], F32, tag="bp")
            b1et = wp_.tile([128, 24], F32, tag="b1e")
            b2t = wp_.tile([128, 4], F32, tag="b2")
            hb3t = wp_.tile([128, 4], F32, tag="hb3")
            nc.gpsimd.dma_start(bpt[:], bpd[:])
            w1t = wp_.tile([128, KB, 512], F8, tag="w1")
            w2t = wp_.tile([128, KB, 512], F8, tag="w2")
            w3t = wp_.tile([128, KB, 512], F8, tag="w3")

            def load_weights():
                for kb in range(KB):
                    nc.gpsimd.dma_start(w1t[:, kb, :],
                                        w1d[kb * 128:(kb + 1) * 128, :])
                nc.sync.dma_start(b1et[:], b1ed[:])
                for kb in range(KB):
                    nc.sync.dma_start(w2t[:, kb, :],
                                      w2d[kb * 128:(kb + 1) * 128, :])
                nc.sync.dma_start(b2t[:], b2d[:])
                for kb in range(KB):
                    nc.gpsimd.dma_start(w3t[:, kb, :],
                                        w3d[kb * 128:(kb + 1) * 128, :])
                nc.sync.dma_start(hb3t[:], hb3d[:])

            # ---- persistent per-half state ----
            s = sp.tile([128, KB, HALF], F32, tag="s")
            s8 = sp.tile([128, KB, HALF], F8, tag="s8")
            cf = sp.tile([128, KB, HALF], F16, tag="cf")
            k8 = sp.tile([128, KB, HALF], F8, tag="k8")
            h1 = sp.tile([128, KB, HALF], F8, tag="h1")
            h2 = sp.tile([128, KB, HALF], F8, tag="h2")
            P = {j: sp.tile([128, KB, HALF], F16, tag=f"P{j}", name=f"P{j}")
                 for j in range(3, 7)}
            X = {j: sp.tile([128, KB, HALF], F8, tag=f"X{j}", name=f"X{j}")
                 for j in range(2, 7)}
            uTl = sp.tile([128, 2 * KBP, HALF], F16, tag="uTl")

            def eng(name):
                return nc.gpsimd if name == "pool" else nc.vector

            for half in range(2):
                c0 = half * HALF
                for kb in range(2 * KBP):
                    e = nc.gpsimd if kb % 2 == 0 else nc.sync
                    e.dma_start(uTl[:, kb, :],
                                uT[kb * 128:(kb + 1) * 128, c0:c0 + HALF])
                for kb in range(KB):
                    e = nc.gpsimd if kb % 2 == 0 else nc.sync
                    e.dma_start(s[:, kb, :],
                                yT[kb * 128:(kb + 1) * 128, c0:c0 + HALF])
                if half == 0:
                    load_weights()

                # input projection: s = y + u @ Wp + bp (fp16 hi/lo matmuls)
                pairs = [(0, 0), (1, 1), (2, 0), (3, 1), (0, 2), (1, 3)]
                for mb in range(4):
                    acc = pp.tile([128, HALF], F32, tag="psum", name="acc")
                    for pi, (ub, wb) in enumerate(pairs):
                        lhsT = wpt[:, wb * 512 + mb * 128:wb * 512 + (mb + 1) * 128]
                        for c in range(CPH):
                            nc.tensor.matmul(
                                acc[:, c * NC_CHUNK:(c + 1) * NC_CHUNK], lhsT,
                                uTl[:, ub, c * NC_CHUNK:(c + 1) * NC_CHUNK],
                                start=(pi == 0), stop=(pi == len(pairs) - 1))
                    nc.vector.scalar_tensor_tensor(
                        s[:, mb, :], acc[:], bpt[:, mb:mb + 1], s[:, mb, :],
                        op0=OP.add, op1=OP.add)
                    nc.gpsimd.tensor_copy(s8[:, mb, :], s[:, mb, :])

                for _step in range(n_steps):
                    last = _step == n_steps - 1
                    for st in range(6):
                        x = s8 if st == 0 else X[st + 1]
                        jb = st  # layer-1 bias variant for this stage

                        def drain1(mb, acc, jb=jb):
                            nc.scalar.activation(
                                h1[:, mb, :], acc[:], AF.Tanh,
                                bias=b1et[:, jb * 4 + mb:jb * 4 + mb + 1],
                                scale=1.0 / WS)

                        def drain2(mb, acc):
                            nc.scalar.activation(
                                h2[:, mb, :], acc[:], AF.Tanh,
                                bias=b2t[:, mb:mb + 1], scale=1.0 / WS)

                        def drain3(mb, acc):
                            nc.vector.tensor_scalar_mul(
                                k8[:, mb, :], acc[:], 1.0 / WS)

                        _dr_layer(nc, pp, w1t, x, drain1)
                        _dr_layer(nc, pp, w2t, h1, drain2)
                        _dr_layer(nc, pp, w3t, h2, drain3)

                        if st < 5:
                            # gating: x_{st+2} = c*k + src, chunked on DVE so
                            # the next stage's matmuls start after block 0
                            cg = float(H * _A[st][st])
                            srcg = s if st == 0 else P[st + 2]
                            xn = X[st + 2]
                            for c in range(CPH):
                                cs = slice(c * NC_CHUNK, (c + 1) * NC_CHUNK)
                                for b in range(KB):
                                    nc.vector.scalar_tensor_tensor(
                                        xn[:, b:b + 1, cs], k8[:, b:b + 1, cs],
                                        cg, srcg[:, b:b + 1, cs],
                                        op0=OP.mult, op1=OP.add)
                            # scatters into future P partials
                            for j in range(st + 3, 7):
                                cj = float(H * _A[j - 2][st])
                                src = s if st == 0 else P[j]
                                eng(_SCATTER_ENG[(st, j)]).scalar_tensor_tensor(
                                    P[j][:], k8[:], cj, src[:],
                                    op0=OP.mult, op1=OP.add)
                            # final-combination accumulator
                            if st == 0:
                                eng(_CF_ENG[0]).tensor_scalar_mul(
                                    cf[:], k8[:], float(H * _B[0]))
                            elif _B[st] != 0.0:
                                eng(_CF_ENG[st]).scalar_tensor_tensor(
                                    cf[:], k8[:], float(H * _B[st]), cf[:],
                                    op0=OP.mult, op1=OP.add)
                        else:
                            # stage 6: cf += hB6*k6 (chunked per block), then
                            # s8_next/s_next = s + cf + h*b3
                            cB6 = float(H * _B[5])
                            for b in range(KB):
                                nc.vector.scalar_tensor_tensor(
                                    cf[:, b:b + 1, :], k8[:, b:b + 1, :],
                                    cB6, cf[:, b:b + 1, :],
                                    op0=OP.mult, op1=OP.add)
                            if not last:
                                for c in range(CPH):
                                    cs = slice(c * NC_CHUNK, (c + 1) * NC_CHUNK)
                                    for b in range(KB):
                                        nc.vector.scalar_tensor_tensor(
                                            s8[:, b:b + 1, cs],
                                            cf[:, b:b + 1, cs],
                                            hb3t[:, b:b + 1],
                                            s[:, b:b + 1, cs],
                                            op0=OP.add, op1=OP.add)
                                for b in range(KB):
                                    nc.vector.scalar_tensor_tensor(
                                        s[:, b:b + 1, :], cf[:, b:b + 1, :],
                                        hb3t[:, b:b + 1], s[:, b:b + 1, :],
                                        op0=OP.add, op1=OP.add)
                            else:
                                # last step: s only, interleave output DMA
                                for q in range(2 * KB):
                                    b, c = divmod(q, CPH)
                                    cs = slice(c * NC_CHUNK, (c + 1) * NC_CHUNK)
                                    nc.vector.scalar_tensor_tensor(
                                        s[:, b:b + 1, cs], cf[:, b:b + 1, cs],
                                        hb3t[:, b:b + 1], s[:, b:b + 1, cs],
                                        op0=OP.add, op1=OP.add)
                                    nc.sync.dma_start(
                                        outT[b * 128:(b + 1) * 128,
                                             c0 + c * NC_CHUNK:
                                             c0 + (c + 1) * NC_CHUNK],
                                        s[:, b:b + 1, cs])

    nc.compile()
    return nc


_NC_CACHE = {}


def _get_nc(n_steps=NSTEPS):
    if n_steps not in _NC_CACHE:
        _NC_CACHE[n_steps] = build_nc(n_steps)
    return _NC_CACHE[n_steps]


def _make_in_maps(inputs):
    y = np.asarray(inputs["y"], np.float32)
    u_t = np.asarray(inputs["u_t"], np.float32)
    yT = np.ascontiguousarray(y.T)
    uT = np.ascontiguousarray(u_t.T)
    wp32 = np.asarray(inputs["Wp"], np.float32)
    wp_hi = wp32.astype(np.float16)
    wp_lo = (wp32 - wp_hi.astype(np.float32)).astype(np.float16)
    uT_hi = uT.astype(np.float16)
    uT_lo = (uT - uT_hi.astype(np.float32)).astype(np.float16)
    uT = np.concatenate([uT_hi, uT_lo], axis=0)

    def q8(w):
        return (WS * np.asarray(w, np.float64)).astype(ml_dtypes.float8_e4m3)

    w1q, w2q, w3q = q8(inputs["W1"]), q8(inputs["W2"]), q8(inputs["W3"])
    w1deq = w1q.astype(np.float64) / WS
    b1 = np.asarray(inputs["b1"], np.float64)
    b2 = np.asarray(inputs["b2"], np.float64)
    b3 = np.asarray(inputs["b3"], np.float64)
    b3w1 = b3 @ w1deq
    # per-stage layer-1 bias: stage st consumes x_{st+1}; x_j (j>=2) is missing
    # beta_j*b3 (beta_j = h * sum of tableau row); x_1 = s is exact.
    b1e = np.zeros((6, HID), np.float64)
    b1e[0] = b1
    for st in range(1, 6):
        beta = H * float(sum(_A[st - 1]))
        b1e[st] = b1 + beta * b3w1
    shared = {
        "w1": np.ascontiguousarray(w1q),
        "w2": np.ascontiguousarray(w2q),
        "w3": np.ascontiguousarray(w3q),
        "wp": np.ascontiguousarray(np.concatenate([wp_hi, wp_lo], axis=0)),
        "bp": np.ascontiguousarray(
            np.asarray(inputs["bp"], np.float32).reshape(4, 128).T),
        "b1e": np.ascontiguousarray(
            b1e.reshape(6, 4, 128).transpose(2, 0, 1).reshape(128, 24)
            .astype(np.float32)),
        "b2": np.ascontiguousarray(b2.astype(np.float32).reshape(4, 128).T),
        "hb3": np.ascontiguousarray(
            (H * b3).astype(np.float32).reshape(4, 128).T),
    }
    in_maps = []
    for i in range(N_CORES):
        sl = slice(i * BSH, (i + 1) * BSH)
        m = dict(shared)
        m["yT"] = np.ascontiguousarray(yT[:, sl])
        m["uT"] = np.ascontiguousarray(uT[:, sl])
        in_maps.append(m)
    return in_maps


def _run(inputs, trace=False, n_steps=NSTEPS):
    nc = _get_nc(n_steps)
    in_maps = _make_in_maps(inputs)
    res = run_bass_kernel_spmd(nc, in_maps, list(range(N_CORES)), trace=trace)
    out = np.empty((HID, B), np.float32)
    for i in range(N_CORES):
        out[:, i * BSH:(i + 1) * BSH] = res.results[i]["outT"]
    return np.ascontiguousarray(out.T), res


def kernel(**inputs) -> np.ndarray:
    out, _ = _run(inputs, trace=False)
    return out


# revision 6
# speedup vs baseline: 1.2909x; 1.2909x over previous
"""Trainium2 Bass kernel for the NeuralODE layer (dopri5 fixed-step, 8 steps).

v4: fp8 DoubleRow matmuls with the RK tableau folded into the tensor engine.

- Inner MLP matmuls in fp8 e4m3 DoubleRow. Weights quantized at scale WS=16,
  activations unscaled fp8, k tiles stored as m/KS (KS=16).
- Stage inputs x_j = s + sum c_jm k_m are mostly formed inside the layer-1
  PSUM accumulation using 8 pre-scaled fp8 W1 copies; only 5 early-term axpys
  remain on DVE (X3/X4/X5 partials).
- The final combination cf = sum h B_m k_m + h b3 is accumulated in the
  stage-6 layer-3 PSUM itself: W3 enters via a copy scaled by C*h*B6,
  k1/k3/k4/k5 via scaled-identity DoubleRow matmuls (B2=0), and h*b3 via a
  K=1 ones-row matmul. One stt per PSUM tile then updates the state:
  s += acc/C. k6 never exists; no cf tile exists.
- b3 folded on host into per-stage layer-1 biases; tanh drains emit fp8.
- Engine roles: ACT = tanh only; DVE = PSUM drains + 5 axpys + state update;
  GpSimd/Sync = DMA queues only (GpSimd tensor ops measured ~14 ns/el: unusable).
"""

import numpy as np
import ml_dtypes

import concourse.bacc as bacc
import concourse.tile as tile
import concourse.mybir as mybir
from concourse.bass_utils import run_bass_kernel_spmd

F32 = mybir.dt.float32
F16 = mybir.dt.float16
F8 = mybir.dt.float8e4
AF = mybir.ActivationFunctionType
OP = mybir.AluOpType
DR = mybir.MatmulPerfMode.DoubleRow

N_CORES = 8
B, IN_DIM, HID = 16384, 256, 512
BSH = B // N_CORES
HALF = 1024
NSTEPS = 8
H = 0.1 * 1 / 8
WS = 16.0                   # fp8 weight scale
KS = 16.0                   # k tiles stored as m/KS

_A = (
    (1 / 5,),
    (3 / 40, 9 / 40),
    (44 / 45, -56 / 15, 32 / 9),
    (19372 / 6561, -25360 / 2187, 64448 / 6561, -212 / 729),
    (9017 / 3168, -355 / 33, 46732 / 5247, 49 / 176, -5103 / 18656),
)
_B = (35 / 384, 0.0, 500 / 1113, 125 / 192, -2187 / 6784, 11 / 84)

KB = HID // 128
KBP = IN_DIM // 128
NC = 512
CPH = HALF // NC

# scaled W1 copies (j, m): k_m rides layer-1 of stage j-1 via fp8(WS*KS*c*W1)
WC_IDX = [(2, 1), (3, 2), (4, 1), (6, 1), (6, 2), (6, 3), (6, 4), (6, 5)]
C_CF = 240.0 / (KS * H * _B[3])   # cf psum scale anchor (max identity = 240)
ONES_V = 0.25                     # ones-row value for the h*b3 matmul


def build_nc(n_steps=NSTEPS):
    nc = bacc.Bacc("TRN2", target_bir_lowering=False, debug=False,
                   num_devices=N_CORES)

    yT = nc.declare_dram_parameter("yT", [HID, BSH], F32, isOutput=False)
    uT = nc.declare_dram_parameter("uT", [2 * IN_DIM, BSH], F16, isOutput=False)
    w1d = nc.declare_dram_parameter("w1", [HID, HID], F8, isOutput=False)
    w2d = nc.declare_dram_parameter("w2", [HID, HID], F8, isOutput=False)
    w3d = nc.declare_dram_parameter("w3", [HID, HID], F8, isOutput=False)
    w3cd = nc.declare_dram_parameter("w3c", [HID, HID], F8, isOutput=False)
    wcd = nc.declare_dram_parameter("wc", [8 * HID, HID], F8, isOutput=False)
    idd = nc.declare_dram_parameter("idw", [4 * 128, 128], F8, isOutput=False)
    hb3wd = nc.declare_dram_parameter("hb3w", [1, HID], F8, isOutput=False)
    wpd = nc.declare_dram_parameter("wp", [2 * IN_DIM, HID], F16, isOutput=False)
    bpd = nc.declare_dram_parameter("bp", [128, 4], F32, isOutput=False)
    b1ed = nc.declare_dram_parameter("b1e", [128, 24], F32, isOutput=False)
    b2d = nc.declare_dram_parameter("b2", [128, 4], F32, isOutput=False)
    outT = nc.declare_dram_parameter("outT", [HID, BSH], F32, isOutput=True)

    with tile.TileContext(nc) as tc:
        with (
            tc.tile_pool(name="wpool", bufs=1) as wp_,
            tc.tile_pool(name="spool", bufs=1) as sp,
            tc.tile_pool(name="pp", bufs=2, space="PSUM") as pp,
        ):
            # ---- resident weights/biases ----
            wpt = wp_.tile([128, 2 * KBP * 512], F16, tag="wp")
            for kb in range(2 * KBP):
                nc.gpsimd.dma_start(wpt[:, kb * 512:(kb + 1) * 512],
                                    wpd[kb * 128:(kb + 1) * 128, :])
            bpt = wp_.tile([128, 4], F32, tag="bp")
            b1et = wp_.tile([128, 24], F32, tag="b1e")
            b2t = wp_.tile([128, 4], F32, tag="b2")
            nc.gpsimd.dma_start(bpt[:], bpd[:])
            w1t = wp_.tile([128, KB, 512], F8, tag="w1")
            w2t = wp_.tile([128, KB, 512], F8, tag="w2")
            w3t = wp_.tile([128, KB, 512], F8, tag="w3")
            w3ct = wp_.tile([128, KB, 512], F8, tag="w3c")
            wct = wp_.tile([128, 8 * KB, 512], F8, tag="wc")
            idp13 = wp_.tile([128, 2, 128], F8, tag="idp13")
            idp45 = wp_.tile([128, 2, 128], F8, tag="idp45")
            hb3wt = wp_.tile([1, HID], F8, tag="hb3w")
            ones8 = wp_.tile([1, HALF], F8, tag="ones8")
            nc.vector.memset(ones8[:], ONES_V)

            def load_weights():
                for kb in range(KB):
                    nc.gpsimd.dma_start(w1t[:, kb, :],
                                        w1d[kb * 128:(kb + 1) * 128, :])
                nc.sync.dma_start(b1et[:], b1ed[:])
                for kb in range(KB):
                    nc.sync.dma_start(w2t[:, kb, :],
                                      w2d[kb * 128:(kb + 1) * 128, :])
                nc.sync.dma_start(b2t[:], b2d[:])
                for kb in range(KB):
                    nc.gpsimd.dma_start(w3t[:, kb, :],
                                        w3d[kb * 128:(kb + 1) * 128, :])
                for kb in range(KB):
                    nc.sync.dma_start(w3ct[:, kb, :],
                                      w3cd[kb * 128:(kb + 1) * 128, :])
                for i in range(8):
                    for kb in range(KB):
                        e = nc.sync if (i * KB + kb) % 2 else nc.gpsimd
                        e.dma_start(wct[:, i * KB + kb, :],
                                    wcd[i * HID + kb * 128:
                                        i * HID + kb * 128 + 128, :])
                nc.gpsimd.dma_start(idp13[:, 0, :], idd[0:128, :])
                nc.gpsimd.dma_start(idp13[:, 1, :], idd[128:256, :])
                nc.sync.dma_start(idp45[:, 0, :], idd[256:384, :])
                nc.sync.dma_start(idp45[:, 1, :], idd[384:512, :])
                nc.sync.dma_start(hb3wt[:], hb3wd[:])

            # ---- persistent per-half state ----
            s = sp.tile([128, KB, HALF], F32, tag="s")
            s8 = sp.tile([128, KB, HALF], F8, tag="s8")
            h1 = sp.tile([128, KB, HALF], F8, tag="h1")
            h2 = sp.tile([128, KB, HALF], F8, tag="h2")
            k8 = sp.tile([128, 5 * KB, HALF], F8, tag="k8")  # dim1=(m-1)*4+kb
            P5 = sp.tile([128, KB, HALF], F16, tag="P5")
            X = {j: sp.tile([128, KB, HALF], F8, tag=f"X{j}", name=f"X{j}")
                 for j in (3, 4, 5)}
            uTl = sp.tile([128, 2 * KBP, HALF], F16, tag="uTl")

            def kv(m):
                return (m - 1) * KB  # k8 dim1 base for k_m

            def layer(groups, drain):
                """groups: (w_tile, w_base, x_tile, x_base); accumulate into
                [128, 2048] accs (mb pairs), then drain(mbi, acc)."""
                for mbi in range(2):
                    acc = pp.tile([128, 2048], F32, tag="psum", name="acc")
                    for m2 in range(2):
                        mb = mbi * 2 + m2
                        for c in range(CPH):
                            out = acc[:, m2 * HALF + c * NC:
                                      m2 * HALF + (c + 1) * NC]
                            ng = len(groups)
                            for gi, (wt, wb, xt, xb) in enumerate(groups):
                                for p in range(2):
                                    nc.tensor.matmul(
                                        out,
                                        wt[:, wb + 2 * p:wb + 2 * p + 2,
                                           mb * 128:(mb + 1) * 128],
                                        xt[:, xb + 2 * p:xb + 2 * p + 2,
                                           c * NC:(c + 1) * NC],
                                        start=(gi == 0 and p == 0),
                                        stop=(gi == ng - 1 and p == 1),
                                        perf_mode=DR)
                    drain(mbi, acc)

            def cf_layer(drain):
                """stage-6 layer 3: psum = W3c-matmul + identity k-terms +
                ones-row h*b3; drain applies s += acc/C."""
                for mbi in range(2):
                    acc = pp.tile([128, 2048], F32, tag="psum", name="acc")
                    for m2 in range(2):
                        mb = mbi * 2 + m2
                        for c in range(CPH):
                            out = acc[:, m2 * HALF + c * NC:
                                      m2 * HALF + (c + 1) * NC]
                            cs = slice(c * NC, (c + 1) * NC)
                            for p in range(2):
                                nc.tensor.matmul(
                                    out,
                                    w3ct[:, 2 * p:2 * p + 2,
                                         mb * 128:(mb + 1) * 128],
                                    h2[:, 2 * p:2 * p + 2, cs],
                                    start=(p == 0), stop=False,
                                    perf_mode=DR)
                            # identity pairs (k1,k3) stride 8, (k4,k5) stride 4
                            nc.tensor.matmul(
                                out, idp13[:],
                                k8[:, mb:2 * KB + mb + 1:2 * KB, cs],
                                start=False, stop=False, perf_mode=DR)
                            nc.tensor.matmul(
                                out, idp45[:],
                                k8[:, 3 * KB + mb:4 * KB + mb + 1:KB, cs],
                                start=False, stop=False, perf_mode=DR)
                            nc.tensor.matmul(
                                out, hb3wt[0:1, mb * 128:(mb + 1) * 128],
                                ones8[0:1, cs],
                                start=False, stop=True)
                    drain(mbi, acc)

            wcb = {jm: i * KB for i, jm in enumerate(WC_IDX)}

            for half in range(2):
                c0 = half * HALF
                for kb in range(2 * KBP):
                    e = nc.gpsimd if kb % 2 == 0 else nc.sync
                    e.dma_start(uTl[:, kb, :],
                                uT[kb * 128:(kb + 1) * 128, c0:c0 + HALF])
                for kb in range(KB):
                    e = nc.gpsimd if kb % 2 == 0 else nc.sync
                    e.dma_start(s[:, kb, :],
                                yT[kb * 128:(kb + 1) * 128, c0:c0 + HALF])
                if half == 0:
                    load_weights()

                # input projection: s = y + u @ Wp + bp (fp16 hi/lo matmuls)
                pairs = [(0, 0), (1, 1), (2, 0), (3, 1), (0, 2), (1, 3)]
                for mbi in range(2):
                    acc = pp.tile([128, 2048], F32, tag="psum", name="acc")
                    for m2 in range(2):
                        mb = mbi * 2 + m2
                        for c in range(CPH):
                            out = acc[:, m2 * HALF + c * NC:
                                      m2 * HALF + (c + 1) * NC]
                            for pi, (ub, wb) in enumerate(pairs):
                                nc.tensor.matmul(
                                    out,
                                    wpt[:, wb * 512 + mb * 128:
                                        wb * 512 + (mb + 1) * 128],
                                    uTl[:, ub, c * NC:(c + 1) * NC],
                                    start=(pi == 0), stop=(pi == len(pairs) - 1))
                    for m2 in range(2):
                        mb = mbi * 2 + m2
                        nc.vector.scalar_tensor_tensor(
                            s[:, mb, :], acc[:, m2 * HALF:(m2 + 1) * HALF],
                            bpt[:, mb:mb + 1], s[:, mb, :],
                            op0=OP.add, op1=OP.add)
                        nc.vector.tensor_copy(s8[:, mb, :], s[:, mb, :])

                for _step in range(n_steps):
                    last = _step == n_steps - 1
                    for st in range(6):
                        if st == 0:
                            g1 = [(w1t, 0, s8, 0)]
                        elif st == 1:
                            g1 = [(w1t, 0, s8, 0), (wct, wcb[(2, 1)], k8, kv(1))]
                        elif st == 2:
                            g1 = [(w1t, 0, X[3], 0), (wct, wcb[(3, 2)], k8, kv(2))]
                        elif st == 3:
                            g1 = [(w1t, 0, X[4], 0), (wct, wcb[(4, 1)], k8, kv(1))]
                        elif st == 4:
                            g1 = [(w1t, 0, X[5], 0)]
                        else:
                            g1 = [(w1t, 0, s8, 0)] + [
                                (wct, wcb[(6, m)], k8, kv(m))
                                for m in range(1, 6)]

                        def drain1(mbi, acc, st=st):
                            for m2 in range(2):
                                mb = mbi * 2 + m2
                                nc.scalar.activation(
                                    h1[:, mb, :],
                                    acc[:, m2 * HALF:(m2 + 1) * HALF],
                                    AF.Tanh,
                                    bias=b1et[:, st * 4 + mb:st * 4 + mb + 1],
                                    scale=1.0 / WS)

                        def drain2(mbi, acc):
                            for m2 in range(2):
                                mb = mbi * 2 + m2
                                nc.scalar.activation(
                                    h2[:, mb, :],
                                    acc[:, m2 * HALF:(m2 + 1) * HALF],
                                    AF.Tanh,
                                    bias=b2t[:, mb:mb + 1], scale=1.0 / WS)

                        layer(g1, drain1)
                        layer([(w2t, 0, h1, 0)], drain2)

                        if st < 5:
                            def drain3(mbi, acc, st=st):
                                nc.vector.tensor_scalar_mul(
                                    k8[:, kv(st + 1) + mbi * 2:
                                       kv(st + 1) + mbi * 2 + 2, :],
                                    acc[:], 1.0 / (WS * KS))
                            layer([(w3t, 0, h2, 0)], drain3)
                        else:
                            def drain3(mbi, acc, last=last, c0=c0):
                                bs = slice(mbi * 2, mbi * 2 + 2)
                                nc.vector.scalar_tensor_tensor(
                                    s[:, bs, :], acc[:], 1.0 / C_CF,
                                    s[:, bs, :], op0=OP.mult, op1=OP.add)
                                for b in range(mbi * 2, mbi * 2 + 2):
                                    for c in range(CPH):
                                        cs = slice(c * NC, (c + 1) * NC)
                                        if not last:
                                            nc.vector.tensor_copy(
                                                s8[:, b:b + 1, cs],
                                                s[:, b:b + 1, cs])
                                        else:
                                            nc.sync.dma_start(
                                                outT[b * 128:(b + 1) * 128,
                                                     c0 + c * NC:
                                                     c0 + (c + 1) * NC],
                                                s[:, b:b + 1, cs])
                            cf_layer(drain3)

                        # remaining elementwise axpys (DVE)
                        if st == 0:
                            nc.vector.scalar_tensor_tensor(
                                X[3][:], k8[:, kv(1):kv(1) + KB, :],
                                float(KS * H * _A[1][0]), s[:],
                                op0=OP.mult, op1=OP.add)
                            nc.vector.scalar_tensor_tensor(
                                P5[:], k8[:, kv(1):kv(1) + KB, :],
                                float(KS * H * _A[3][0]), s[:],
                                op0=OP.mult, op1=OP.add)
                        elif st == 1:
                            nc.vector.scalar_tensor_tensor(
                                X[4][:], k8[:, kv(2):kv(2) + KB, :],
                                float(KS * H * _A[2][1]), s[:],
                                op0=OP.mult, op1=OP.add)
                            nc.vector.scalar_tensor_tensor(
                                P5[:], k8[:, kv(2):kv(2) + KB, :],
                                float(KS * H * _A[3][1]), P5[:],
                                op0=OP.mult, op1=OP.add)
                        elif st == 2:
                            nc.vector.scalar_tensor_tensor(
                                X[5][:], k8[:, kv(3):kv(3) + KB, :],
                                float(KS * H * _A[3][2]), P5[:],
                                op0=OP.mult, op1=OP.add)

    nc.compile()
    return nc


_NC_CACHE = {}


def _get_nc(n_steps=NSTEPS):
    if n_steps not in _NC_CACHE:
        _NC_CACHE[n_steps] = build_nc(n_steps)
    return _NC_CACHE[n_steps]


def _make_in_maps(inputs):
    y = np.asarray(inputs["y"], np.float32)
    u_t = np.asarray(inputs["u_t"], np.float32)
    yT = np.ascontiguousarray(y.T)
    uT = np.ascontiguousarray(u_t.T)
    wp32 = np.asarray(inputs["Wp"], np.float32)
    wp_hi = wp32.astype(np.float16)
    wp_lo = (wp32 - wp_hi.astype(np.float32)).astype(np.float16)
    uT_hi = uT.astype(np.float16)
    uT_lo = (uT - uT_hi.astype(np.float32)).astype(np.float16)
    uT = np.concatenate([uT_hi, uT_lo], axis=0)

    def q8(w):
        return np.asarray(w, np.float64).astype(ml_dtypes.float8_e4m3)

    W1 = np.asarray(inputs["W1"], np.float64)
    W3 = np.asarray(inputs["W3"], np.float64)
    w1q = q8(WS * W1)
    w2q = q8(WS * np.asarray(inputs["W2"], np.float64))
    w3q = q8(WS * W3)
    w3c = q8(C_CF * H * _B[5] * W3)
    wc = np.concatenate(
        [q8(WS * KS * H * _A[j - 2][m - 1] * W1) for (j, m) in WC_IDX], axis=0)
    eye = np.eye(128, dtype=np.float64)
    idw = np.concatenate(
        [q8(C_CF * KS * H * _B[m - 1] * eye) for m in (1, 3, 4, 5)], axis=0)
    b1 = np.asarray(inputs["b1"], np.float64)
    b2 = np.asarray(inputs["b2"], np.float64)
    b3 = np.asarray(inputs["b3"], np.float64)
    hb3w = q8((C_CF * H / ONES_V) * b3).reshape(1, HID)
    b3w1 = b3 @ (w1q.astype(np.float64) / WS)
    b1e = np.zeros((6, HID), np.float64)
    b1e[0] = b1
    for st in range(1, 6):
        b1e[st] = b1 + H * float(sum(_A[st - 1])) * b3w1
    shared = {
        "w1": np.ascontiguousarray(w1q),
        "w2": np.ascontiguousarray(w2q),
        "w3": np.ascontiguousarray(w3q),
        "w3c": np.ascontiguousarray(w3c),
        "wc": np.ascontiguousarray(wc),
        "idw": np.ascontiguousarray(idw),
        "hb3w": np.ascontiguousarray(hb3w),
        "wp": np.ascontiguousarray(np.concatenate([wp_hi, wp_lo], axis=0)),
        "bp": np.ascontiguousarray(
            np.asarray(inputs["bp"], np.float32).reshape(4, 128).T),
        "b1e": np.ascontiguousarray(
            b1e.reshape(6, 4, 128).transpose(2, 0, 1).reshape(128, 24)
            .astype(np.float32)),
        "b2": np.ascontiguousarray(b2.astype(np.float32).reshape(4, 128).T),
    }
    in_maps = []
    for i in range(N_CORES):
        sl = slice(i * BSH, (i + 1) * BSH)
        m = dict(shared)
        m["yT"] = np.ascontiguousarray(yT[:, sl])
        m["uT"] = np.ascontiguousarray(uT[:, sl])
        in_maps.append(m)
    return in_maps


def _run(inputs, trace=False, n_steps=NSTEPS):
    nc = _get_nc(n_steps)
    in_maps = _make_in_maps(inputs)
    res = run_bass_kernel_spmd(nc, in_maps, list(range(N_CORES)), trace=trace)
    out = np.empty((HID, B), np.float32)
    for i in range(N_CORES):
        out[:, i * BSH:(i + 1) * BSH] = res.results[i]["outT"]
    return np.ascontiguousarray(out.T), res


def kernel(**inputs) -> np.ndarray:
    out, _ = _run(inputs, trace=False)
    return out
